# revision 24
# baseline (speedup 1.0000x reference)
"""Trainium2 Bass kernel for nn_DecSwitchedFC (MoE hard routing).

Math (per token b, expert e = y_idx[b]):
    out[b] = x[b] + z[b, e] * (relu(x[b] @ W1[e] + b1[e]) @ W2[e] + b2[e])

Strategy: expert-parallel over 8 NeuronCores, 2 experts per core, with a
fixed per-expert device capacity of 512 tokens.  Tokens beyond an
expert's capacity (~1% of the batch for a uniform router) are computed
exactly on the host in fp32.  Each core gets exactly 1024 device tokens
with zero padding waste.

Device data layout is fully "feature-major" (d or h on partitions,
tokens on the free axis) so no on-device transposes are needed:
    h^T[256, n]  = W1[e]^T(lhsT) @ x^T          (K=1024, 8 chunks)
    o^T[1024, n] = W2[e]^T(lhsT) @ relu(h^T+b1) (K=256, 2 chunks)
All matmuls run in bf16 (~2.4e-3 final rel err); o^T ships back in bf16
and the z-scale + residual are applied on the host in exact fp32.

Schedule (from perfetto/NTFF traces of this kernel):
  - All data rides ONE DRAM blob and ONE queue (Sync) as 6 big DMAs in
    exact consumption order — a queue is strict FIFO, transfers are
    striped over 16 DMA engines (~430 GB/s read steady, slower while
    ramping), and every extra DMA issue costs ~0.7us of queue time that
    delays everything behind it.
  - DMA#1 fuses [w1-s0 | x-t0] so a single completion event unblocks
    the whole first tile; tiles t0/t1 are 256 tokens so their compute
    covers the supply latency of the later transfers; t2 is 512 wide so
    its 32 LDWEIGHTS (~92ns each) hide behind 213ns matmuls.
  - A chain of warmup matmuls on a memset tile runs while the first
    DMAs are in flight: the PE p-state ramps 1.2->2.4 GHz only after
    ~3us of CONTINUOUS busy (any stall resets it), and a busy queue
    also absorbs the ~1.7us DMA-completion wake latency.
  - DRAM writes run ~2.5x slower than reads (~170 GB/s); the last
    tile's output ships in i-pair chunks so only ~0.26 MB trails the
    final copy.
  - FC1 relu+bias on Scalar; the 8 FC2 bias-add copies per tile split
    Vector/Scalar so neither engine bottlenecks.
"""

import ml_dtypes
import numpy as np

import concourse.bacc as bacc
import concourse.mybir as mybir
import concourse.tile as tile
from concourse.bass_utils import run_bass_kernel_spmd

D = 1024        # model dim
H = 256         # bottleneck dim
NB = 16         # n experts
NCORES = 8
CAP = 512       # device tokens per expert
KC1 = D // 128  # contraction chunks for x @ W1 (8)
KC2 = H // 128  # contraction chunks for h @ W2 (2)
F32 = mybir.dt.float32
BF16 = mybir.dt.bfloat16
NWARM = 14      # warmup matmuls (512 wide): ~6us of PE ramp coverage

# Token tiles: (slot, t0, tn).  Small first tiles keep the PE start
# early (first DMA chunk is 1 MB); the 512-wide last tile halves its
# LDWEIGHTS pressure (32 loads hide behind 213ns matmuls).
TILES = [(0, 0, 256), (0, 256, 256), (1, 0, 512)]

# Input blob column layout (bf16, k-major x packing: col = k*tn + c).
# Order = transfer order = consumption order.
SEC = {}
_off = 0
for _name, _w in [("w1s0", 2048), ("x0", 2048), ("w2s0", 2048),
                  ("x1", 2048), ("x2", 4096), ("w1s1", 2048),
                  ("w2s1", 2048)]:
    SEC[_name] = (_off, _off + _w)
    _off += _w
BLOBCOLS = _off          # 16384
OUTCOLS = KC1 * 2 * CAP  # 8192

_build_cache: dict[tuple, object] = {}
LAST_RESULTS = None  # BassKernelResults of the most recent run (for profiling)


def _build():
    key = ("p2",)
    if key in _build_cache:
        return _build_cache[key]

    nc = bacc.Bacc("TRN2", target_bir_lowering=False, debug=False)

    blob = nc.dram_tensor("blob", [128, BLOBCOLS], BF16, kind="ExternalInput")
    # bias[p, s, j]     = b1[e, 128j + p]   (j in 0..1)
    # bias[p, s, 2 + i] = b2[e, 128i + p]   (i in 0..7)
    bias = nc.dram_tensor("bias", [128, 2, KC2 + KC1], F32,
                          kind="ExternalInput")
    outP = nc.dram_tensor("outP", [128, OUTCOLS], BF16, kind="ExternalOutput")

    with tile.TileContext(nc) as tc:
        with (
            tc.tile_pool(name="const", bufs=1) as cpool,
            tc.tile_pool(name="inp", bufs=1) as ipool,
            tc.tile_pool(name="hp", bufs=2) as hpool,
            tc.tile_pool(name="op", bufs=2) as opool,
            tc.tile_pool(name="ph", bufs=2, space="PSUM") as phpool,
            tc.tile_pool(name="po", bufs=4, space="PSUM") as popool,
            tc.tile_pool(name="pw", bufs=1, space="PSUM") as pwpool,
        ):
            # ---- PE warmup on a memset tile while DMAs are in flight
            warm = cpool.tile([128, 512], BF16, name="warm")
            nc.gpsimd.memset(warm[:], 0)
            pwarm = pwpool.tile([128, 512], F32, name="pwarm")
            for w in range(NWARM):
                nc.tensor.matmul(pwarm[:], warm[:, :128], warm[:],
                                 start=(w == 0), stop=(w == NWARM - 1))

            # ---- input DMAs: bias first (tiny; the relu/copy chain
            # depends on it), then consumption-ordered blob chunks.
            bias_t = cpool.tile([128, 2, KC2 + KC1], F32)
            nc.sync.dma_start(bias_t[:], bias[:])

            def load(name, *secs):
                lo = SEC[secs[0]][0]
                hi = SEC[secs[-1]][1]
                t = ipool.tile([128, hi - lo], BF16, name=name)
                nc.sync.dma_start(t[:], blob[:, lo:hi])
                return t

            wx0 = load("wx0", "w1s0", "x0")       # 1 MB
            w2s0t = load("w2s0t", "w2s0")         # 0.5 MB
            x1t = load("x1t", "x1")               # 0.5 MB
            xw1 = load("xw1", "x2", "w1s1")       # 1.5 MB
            w2s1t = load("w2s1t", "w2s1")         # 0.5 MB

            # per-tile views: w1(s)[j,k] -> [128,128], x(q)[k] -> [128,tn]
            def w1ap(s, j, k):
                t, base = (wx0, 0) if s == 0 else (xw1, 4096)
                return t[:, base + 1024 * j + 128 * k:
                         base + 1024 * j + 128 * (k + 1)]

            def w2ap(s, j, i):
                t = w2s0t if s == 0 else w2s1t
                return t[:, 1024 * j + 128 * i:1024 * j + 128 * (i + 1)]

            def xap(q, k, tn):
                t, base = [(wx0, 2048), (x1t, 0), (xw1, 0)][q]
                return t[:, base + tn * k:base + tn * (k + 1)]

            # ---- compute ----
            xoff = 0
            for q, (s, t0, tn) in enumerate(TILES):
                ht = hpool.tile([128, KC2, tn], BF16, tag="ht")
                for j in range(KC2):
                    ph = phpool.tile([128, tn], F32, tag="ph")
                    for k in range(KC1):
                        nc.tensor.matmul(
                            ph[:], w1ap(s, j, k), xap(q, k, tn),
                            start=(k == 0), stop=(k == KC1 - 1))
                    nc.scalar.activation(
                        ht[:, j, :], ph[:],
                        mybir.ActivationFunctionType.Relu,
                        bias=bias_t[:, s, j:j + 1])

                ot = opool.tile([128, KC1, tn], BF16, tag="ot")
                last = q == len(TILES) - 1
                for i in range(KC1):
                    po = popool.tile([128, tn], F32, tag="po")
                    for j in range(KC2):
                        nc.tensor.matmul(
                            po[:], w2ap(s, j, i), ht[:, j, :],
                            start=(j == 0), stop=(j == KC2 - 1))
                    bcol = bias_t[:, s, KC2 + i:KC2 + i + 1]
                    if i % 3 == 1:
                        nc.scalar.activation(
                            ot[:, i, :], po[:],
                            mybir.ActivationFunctionType.Identity,
                            bias=bcol)
                    else:
                        nc.vector.tensor_scalar_add(ot[:, i, :], po[:], bcol)
                    # DRAM writes run ~2.5x slower than reads; ship the
                    # last tile's output in i-pair chunks, and the final
                    # two i-chunks singly so only ~0.13 MB trails the
                    # final copy.
                    if last and i % 2 == 1 and i < KC1 - 2:
                        i0 = i - 1
                        nc.sync.dma_start(
                            outP[:, xoff + i0 * tn:
                                 xoff + (i + 1) * tn].rearrange(
                                "p (k c) -> p k c", k=2),
                            ot[:, i0:i + 1, :])
                    elif last and i >= KC1 - 2:
                        nc.sync.dma_start(
                            outP[:, xoff + i * tn:
                                 xoff + (i + 1) * tn].rearrange(
                                "p (k c) -> p k c", k=1),
                            ot[:, i:i + 1, :])
                if last:
                    pass
                else:
                    nc.sync.dma_start(
                        outP[:, xoff:xoff + KC1 * tn].rearrange(
                            "p (k c) -> p k c", k=KC1),
                        ot[:])
                xoff += KC1 * tn

    nc.compile()
    _build_cache[key] = nc
    return nc


def kernel(x, y_idx, y, z, W1, b1, W2, b2):
    x = np.ascontiguousarray(np.asarray(x, dtype=np.float32))
    z = np.asarray(z, dtype=np.float32)
    W1 = np.asarray(W1, dtype=np.float32)
    b1 = np.asarray(b1, dtype=np.float32)
    W2 = np.asarray(W2, dtype=np.float32)
    b2 = np.asarray(b2, dtype=np.float32)
    e = np.asarray(y_idx).reshape(-1).astype(np.int64)
    B = x.shape[0]

    idxs = [np.flatnonzero(e == k) for k in range(NB)]

    nc = _build()

    in_maps = []
    for c in range(NCORES):
        blob = np.zeros((128, BLOBCOLS), ml_dtypes.bfloat16)
        bias = np.empty((128, 2, KC2 + KC1), np.float32)
        for s in range(2):
            k = 2 * c + s
            # w1 cols j*1024 + kk*128 + m = W1[k][128kk + p, 128j + m]
            lo = SEC[f"w1s{s}"][0]
            blob[:, lo:lo + 2048] = W1[k].reshape(
                KC1, 128, KC2, 128).transpose(1, 2, 0, 3).reshape(
                128, 2048).astype(ml_dtypes.bfloat16)
            # w2 cols j*1024 + i*128 + m = W2[k][128j + p, 128i + m]
            lo = SEC[f"w2s{s}"][0]
            blob[:, lo:lo + 2048] = W2[k].reshape(
                KC2, 128, KC1, 128).transpose(1, 0, 2, 3).reshape(
                128, 2048).astype(ml_dtypes.bfloat16)
            bias[:, s, :KC2] = b1[k].reshape(KC2, 128).T
            bias[:, s, KC2:] = b2[k].reshape(KC1, 128).T
        for q, (s, t0, tn) in enumerate(TILES):
            k = 2 * c + s
            seg = idxs[k][t0:t0 + tn]
            n = len(seg)
            if n:
                lo = SEC[f"x{q}"][0]
                full = np.zeros((128, KC1, tn), ml_dtypes.bfloat16)
                full[:, :, :n] = x[seg].reshape(
                    n, KC1, 128).transpose(2, 1, 0).astype(ml_dtypes.bfloat16)
                blob[:, lo:lo + KC1 * tn] = full.reshape(128, KC1 * tn)
        in_maps.append({"blob": blob, "bias": bias})

    res = run_bass_kernel_spmd(nc, in_maps, core_ids=list(range(NCORES)))
    global LAST_RESULTS
    LAST_RESULTS = res

    out = np.empty((B, D), np.float32)
    for c in range(NCORES):
        outP = res.results[c]["outP"]
        xoff = 0
        for s, t0, tn in TILES:
            k = 2 * c + s
            seg = idxs[k][t0:t0 + tn]
            n = len(seg)
            if n:
                blk = outP[:, xoff:xoff + KC1 * tn].reshape(128, KC1, tn)
                # blk[p, i, c] = o[token c, 128i + p]
                rows = blk[:, :, :n].transpose(2, 1, 0).reshape(
                    n, D).astype(np.float32)
                out[seg] = x[seg] + z[seg, k][:, None] * rows
            xoff += KC1 * tn

    # Overflow tokens beyond the per-expert device capacity: exact host
    # fp32 compute (~1% of the batch for a uniform router).
    for k in range(NB):
        seg = idxs[k][CAP:]
        if len(seg) == 0:
            continue
        h = np.maximum(x[seg] @ W1[k] + b1[k], 0.0)
        o = h @ W2[k] + b2[k]
        out[seg] = x[seg] + z[seg, k][:, None] * o
    return out


# revision 25
# speedup vs baseline: 1.0197x; 1.0197x over previous
"""Trainium2 Bass kernel for nn_DecSwitchedFC (MoE hard routing).

Math (per token b, expert e = y_idx[b]):
    out[b] = x[b] + z[b, e] * (relu(x[b] @ W1[e] + b1[e]) @ W2[e] + b2[e])

Strategy: expert-parallel over 8 NeuronCores, 2 experts per core, with a
fixed per-expert device capacity of 512 tokens.  Tokens beyond an
expert's capacity (~1% of the batch for a uniform router) are computed
exactly on the host in fp32.  Each core gets exactly 1024 device tokens
with zero padding waste.

Device data layout is fully "feature-major" (d or h on partitions,
tokens on the free axis) so no on-device transposes are needed:
    h^T[256, n]  = W1[e]^T(lhsT) @ x^T          (K=1024, 8 chunks)
    o^T[1024, n] = W2[e]^T(lhsT) @ relu(h^T+b1) (K=256, 2 chunks)
All matmuls run in bf16 (~2.4e-3 final rel err); o^T ships back in bf16
and the z-scale + residual are applied on the host in exact fp32.

Schedule (from perfetto/NTFF traces of this kernel):
  - All data rides ONE DRAM blob and ONE queue (Sync) as 6 big DMAs in
    exact consumption order — a queue is strict FIFO, transfers are
    striped over 16 DMA engines (~430 GB/s read steady, slower while
    ramping), and every extra DMA issue costs ~0.7us of queue time that
    delays everything behind it.
  - DMA#1 fuses [w1-s0 | x-t0] so a single completion event unblocks
    the whole first tile; tiles t0/t1 are 256 tokens so their compute
    covers the supply latency of the later transfers; t2 is 512 wide so
    its 32 LDWEIGHTS (~92ns each) hide behind 213ns matmuls.
  - A chain of warmup matmuls on a memset tile runs while the first
    DMAs are in flight: the PE p-state ramps 1.2->2.4 GHz only after
    ~3us of CONTINUOUS busy (any stall resets it), and a busy queue
    also absorbs the ~1.7us DMA-completion wake latency.
  - DRAM writes run ~2.5x slower than reads (~170 GB/s); the last
    tile's output ships in i-pair chunks so only ~0.26 MB trails the
    final copy.
  - FC1 relu+bias on Scalar; the 8 FC2 bias-add copies per tile split
    Vector/Scalar so neither engine bottlenecks.
"""

import ml_dtypes
import numpy as np

import concourse.bacc as bacc
import concourse.mybir as mybir
import concourse.tile as tile
from concourse.bass_utils import run_bass_kernel_spmd

D = 1024        # model dim
H = 256         # bottleneck dim
NB = 16         # n experts
NCORES = 8
CAP = 512       # device tokens per expert
KC1 = D // 128  # contraction chunks for x @ W1 (8)
KC2 = H // 128  # contraction chunks for h @ W2 (2)
F32 = mybir.dt.float32
BF16 = mybir.dt.bfloat16
NWARM = 15      # warmup matmuls (512 wide): ~6.5us of PE ramp coverage

# Token tiles: (slot, t0, tn).  Small first tiles keep the PE start
# early (first DMA chunk is 1 MB); the 512-wide last tile halves its
# LDWEIGHTS pressure (32 loads hide behind 213ns matmuls).
TILES = [(0, 0, 256), (0, 256, 256), (1, 0, 512)]

# Input blob column layout (bf16, k-major x packing: col = k*tn + c).
# Order = transfer order = consumption order.
SEC = {}
_off = 0
for _name, _w in [("w1s0", 2048), ("x0", 2048), ("w2s0", 2048),
                  ("x1", 2048), ("x2", 4096), ("w1s1", 2048),
                  ("w2s1", 2048)]:
    SEC[_name] = (_off, _off + _w)
    _off += _w
BLOBCOLS = _off          # 16384
OUTCOLS = KC1 * 2 * CAP  # 8192

_build_cache: dict[tuple, object] = {}
LAST_RESULTS = None  # BassKernelResults of the most recent run (for profiling)


def _build():
    key = ("final2",)
    if key in _build_cache:
        return _build_cache[key]

    nc = bacc.Bacc("TRN2", target_bir_lowering=False, debug=False)

    blob = nc.dram_tensor("blob", [128, BLOBCOLS], BF16, kind="ExternalInput")
    # bias[p, s, j]     = b1[e, 128j + p]   (j in 0..1)
    # bias[p, s, 2 + i] = b2[e, 128i + p]   (i in 0..7)
    bias = nc.dram_tensor("bias", [128, 2, KC2 + KC1], F32,
                          kind="ExternalInput")
    outP = nc.dram_tensor("outP", [128, OUTCOLS], BF16, kind="ExternalOutput")

    with tile.TileContext(nc) as tc:
        with (
            tc.tile_pool(name="const", bufs=1) as cpool,
            tc.tile_pool(name="inp", bufs=1) as ipool,
            tc.tile_pool(name="hp", bufs=2) as hpool,
            tc.tile_pool(name="op", bufs=2) as opool,
            tc.tile_pool(name="ph", bufs=2, space="PSUM") as phpool,
            tc.tile_pool(name="po", bufs=4, space="PSUM") as popool,
            tc.tile_pool(name="pw", bufs=1, space="PSUM") as pwpool,
        ):
            # ---- PE warmup on a memset tile while DMAs are in flight
            warm = cpool.tile([128, 512], BF16, name="warm")
            nc.gpsimd.memset(warm[:], 0)
            pwarm = pwpool.tile([128, 512], F32, name="pwarm")
            for w in range(NWARM):
                nc.tensor.matmul(pwarm[:], warm[:, :128], warm[:],
                                 start=(w == 0), stop=(w == NWARM - 1))

            # ---- input DMAs: bias first (tiny; the relu/copy chain
            # depends on it), then consumption-ordered blob chunks.
            bias_t = cpool.tile([128, 2, KC2 + KC1], F32)
            nc.sync.dma_start(bias_t[:], bias[:])

            def load(name, *secs):
                lo = SEC[secs[0]][0]
                hi = SEC[secs[-1]][1]
                t = ipool.tile([128, hi - lo], BF16, name=name)
                nc.sync.dma_start(t[:], blob[:, lo:hi])
                return t

            wx0 = load("wx0", "w1s0", "x0")       # 1 MB
            w2s0t = load("w2s0t", "w2s0")         # 0.5 MB
            x1t = load("x1t", "x1")               # 0.5 MB
            xw1 = load("xw1", "x2", "w1s1")       # 1.5 MB
            w2s1t = load("w2s1t", "w2s1")         # 0.5 MB

            # per-tile views: w1(s)[j,k] -> [128,128], x(q)[k] -> [128,tn]
            def w1ap(s, j, k):
                t, base = (wx0, 0) if s == 0 else (xw1, 4096)
                return t[:, base + 1024 * j + 128 * k:
                         base + 1024 * j + 128 * (k + 1)]

            def w2ap(s, j, i):
                t = w2s0t if s == 0 else w2s1t
                return t[:, 1024 * j + 128 * i:1024 * j + 128 * (i + 1)]

            def xap(q, k, tn):
                t, base = [(wx0, 2048), (x1t, 0), (xw1, 0)][q]
                return t[:, base + tn * k:base + tn * (k + 1)]

            # ---- compute ----
            xoff = 0
            for q, (s, t0, tn) in enumerate(TILES):
                ht = hpool.tile([128, KC2, tn], BF16, tag="ht")
                for j in range(KC2):
                    ph = phpool.tile([128, tn], F32, tag="ph")
                    for k in range(KC1):
                        nc.tensor.matmul(
                            ph[:], w1ap(s, j, k), xap(q, k, tn),
                            start=(k == 0), stop=(k == KC1 - 1))
                    nc.scalar.activation(
                        ht[:, j, :], ph[:],
                        mybir.ActivationFunctionType.Relu,
                        bias=bias_t[:, s, j:j + 1])

                ot = opool.tile([128, KC1, tn], BF16, tag="ot")
                last = q == len(TILES) - 1
                for i in range(KC1):
                    po = popool.tile([128, tn], F32, tag="po")
                    for j in range(KC2):
                        nc.tensor.matmul(
                            po[:], w2ap(s, j, i), ht[:, j, :],
                            start=(j == 0), stop=(j == KC2 - 1))
                    bcol = bias_t[:, s, KC2 + i:KC2 + i + 1]
                    if i % 3 == 1:
                        nc.scalar.activation(
                            ot[:, i, :], po[:],
                            mybir.ActivationFunctionType.Identity,
                            bias=bcol)
                    else:
                        nc.vector.tensor_scalar_add(ot[:, i, :], po[:], bcol)
                    # DRAM writes run ~2.5x slower than reads; ship the
                    # last tile's output in i-pair chunks so only a
                    # 0.26 MB chunk trails the final copy.
                    if last and i % 2 == 1 and i < KC1 - 1:
                        i0 = i - 1
                        nc.sync.dma_start(
                            outP[:, xoff + i0 * tn:
                                 xoff + (i + 1) * tn].rearrange(
                                "p (k c) -> p k c", k=2),
                            ot[:, i0:i + 1, :])
                if last:
                    nc.sync.dma_start(
                        outP[:, xoff + (KC1 - 2) * tn:
                             xoff + KC1 * tn].rearrange(
                            "p (k c) -> p k c", k=2),
                        ot[:, KC1 - 2:, :])
                else:
                    nc.sync.dma_start(
                        outP[:, xoff:xoff + KC1 * tn].rearrange(
                            "p (k c) -> p k c", k=KC1),
                        ot[:])
                xoff += KC1 * tn

    nc.compile()
    _build_cache[key] = nc
    return nc


def kernel(x, y_idx, y, z, W1, b1, W2, b2):
    x = np.ascontiguousarray(np.asarray(x, dtype=np.float32))
    z = np.asarray(z, dtype=np.float32)
    W1 = np.asarray(W1, dtype=np.float32)
    b1 = np.asarray(b1, dtype=np.float32)
    W2 = np.asarray(W2, dtype=np.float32)
    b2 = np.asarray(b2, dtype=np.float32)
    e = np.asarray(y_idx).reshape(-1).astype(np.int64)
    B = x.shape[0]

    idxs = [np.flatnonzero(e == k) for k in range(NB)]

    nc = _build()

    in_maps = []
    for c in range(NCORES):
        blob = np.zeros((128, BLOBCOLS), ml_dtypes.bfloat16)
        bias = np.empty((128, 2, KC2 + KC1), np.float32)
        for s in range(2):
            k = 2 * c + s
            # w1 cols j*1024 + kk*128 + m = W1[k][128kk + p, 128j + m]
            lo = SEC[f"w1s{s}"][0]
            blob[:, lo:lo + 2048] = W1[k].reshape(
                KC1, 128, KC2, 128).transpose(1, 2, 0, 3).reshape(
                128, 2048).astype(ml_dtypes.bfloat16)
            # w2 cols j*1024 + i*128 + m = W2[k][128j + p, 128i + m]
            lo = SEC[f"w2s{s}"][0]
            blob[:, lo:lo + 2048] = W2[k].reshape(
                KC2, 128, KC1, 128).transpose(1, 0, 2, 3).reshape(
                128, 2048).astype(ml_dtypes.bfloat16)
            bias[:, s, :KC2] = b1[k].reshape(KC2, 128).T
            bias[:, s, KC2:] = b2[k].reshape(KC1, 128).T
        for q, (s, t0, tn) in enumerate(TILES):
            k = 2 * c + s
            seg = idxs[k][t0:t0 + tn]
            n = len(seg)
            if n:
                lo = SEC[f"x{q}"][0]
                full = np.zeros((128, KC1, tn), ml_dtypes.bfloat16)
                full[:, :, :n] = x[seg].reshape(
                    n, KC1, 128).transpose(2, 1, 0).astype(ml_dtypes.bfloat16)
                blob[:, lo:lo + KC1 * tn] = full.reshape(128, KC1 * tn)
        in_maps.append({"blob": blob, "bias": bias})

    res = run_bass_kernel_spmd(nc, in_maps, core_ids=list(range(NCORES)))
    global LAST_RESULTS
    LAST_RESULTS = res

    out = np.empty((B, D), np.float32)
    for c in range(NCORES):
        outP = res.results[c]["outP"]
        xoff = 0
        for s, t0, tn in TILES:
            k = 2 * c + s
            seg = idxs[k][t0:t0 + tn]
            n = len(seg)
            if n:
                blk = outP[:, xoff:xoff + KC1 * tn].reshape(128, KC1, tn)
                # blk[p, i, c] = o[token c, 128i + p]
                rows = blk[:, :, :n].transpose(2, 1, 0).reshape(
                    n, D).astype(np.float32)
                out[seg] = x[seg] + z[seg, k][:, None] * rows
            xoff += KC1 * tn

    # Overflow tokens beyond the per-expert device capacity: exact host
    # fp32 compute (~1% of the batch for a uniform router).
    for k in range(NB):
        seg = idxs[k][CAP:]
        if len(seg) == 0:
            continue
        h = np.maximum(x[seg] @ W1[k] + b1[k], 0.0)
        o = h @ W2[k] + b2[k]
        out[seg] = x[seg] + z[seg, k][:, None] * o
    return out
